# revision 17
# baseline (speedup 1.0000x reference)
"""Trainium2 Bass kernel for causal multi-head attention block.

Reference computation (B=4, S=2048, D=1024, H=16, HD=64, fp32):
    qkv = x @ Wqkv + bqkv; split q,k,v; per-head scaled scores;
    causal mask filled with -0.0001 (leaky, NOT -inf); softmax over all
    2048 keys; out = P @ V; out = out @ Wo + bo.

Sharding: 8 cores, core = (batch b = i//2, parity p = i%2). Each core
computes 1024 queries of its batch: query tiles t = 2j+p (j=0..3) of
256 queries; qtile j needs key blocks 0..j on every core -> one SPMD
program, zero cross-core communication.

v4 design notes:
  - All matmuls 512-wide moving operands where possible (the PE
    weight-slot recycle makes narrower MMs latency-bound).
  - Scores for both heads of a pair go to one [128,2,512] PSUM tile ->
    ONE exp activation per (kb,s2) unit ([128,1024]); ACT per-call
    overhead is 352 cycles so bigger calls matter.
  - The scalar engine (exp) and tensor engine run a tight
    producer/consumer loop in attention; projection matmuls for the
    NEXT pair are emitted interleaved into the attention stream (fill
    queue) so the PE never idles and HAM stays at K=8/8.
  - Elementwise bias/copy work moved to the idle GpSimd (Pool) engine;
    DVE keeps copy_predicated (mask), reciprocal, and the epilogue.
  - Leaky causal mask: e = mask ? w : exp(S) via one copy_predicated
    per diag unit (mask duplicated per head on host).
  - Z denominator via 65th all-ones V column; numerator correction for
    skipped key blocks via W-scaled suffix sums of per-block V sums.
"""

import math
from collections import deque
from contextlib import ExitStack

import numpy as np

import concourse.bass as bass
import concourse.mybir as mybir
import concourse.tile as tile
from concourse import bacc

F32 = mybir.dt.float32
F32R = mybir.dt.float32r
BF16 = mybir.dt.bfloat16
U8 = mybir.dt.uint8
AF = mybir.ActivationFunctionType
ALU = mybir.AluOpType
AX = mybir.AxisListType

B, S, D, H, HD = 4, 2048, 1024, 16, 64
QL, QT, KB, NJ = 1024, 256, 512, 4
NCH = D // 128
PAIRS = H // 2
W_MASK = math.exp(-1e-4)


def build_program():
    nc = bacc.Bacc(
        "TRN2",
        target_bir_lowering=False,
        debug=False,
        num_devices=8,
    )
    xq = nc.declare_dram_parameter("xq", [128, NCH, QL], BF16, isOutput=False)
    xt = nc.declare_dram_parameter("xt", [128, NCH, S], BF16, isOutput=False)
    wq = nc.declare_dram_parameter("wq", [128, NCH, D], BF16, isOutput=False)
    wk = nc.declare_dram_parameter("wk", [128, NCH, D], BF16, isOutput=False)
    wv = nc.declare_dram_parameter("wv", [128, NCH, D], BF16, isOutput=False)
    wo = nc.declare_dram_parameter("wo", [128, NCH, D], BF16, isOutput=False)
    b2h = nc.declare_dram_parameter("b2h", [128, 16], F32, isOutput=False)
    brow = nc.declare_dram_parameter("brow", [1, D], BF16, isOutput=False)
    bv512 = nc.declare_dram_parameter("bv512", [128, 8], F32, isOutput=False)
    bocol = nc.declare_dram_parameter("bocol", [128, 8], F32, isOutput=False)
    mdup = nc.declare_dram_parameter("mdup", [128, 8 * QT], U8, isOutput=False)
    onesd = nc.declare_dram_parameter("onesd", [1, 64], F32R, isOutput=False)
    outT = nc.declare_dram_parameter("outT", [D, QL], F32, isOutput=True)

    with tile.TileContext(nc) as tc, ExitStack() as ctx, \
         nc.allow_low_precision(reason="bf16 compute, tolerance 2e-2"):
        consts = ctx.enter_context(tc.tile_pool(name="consts", bufs=1))
        ones_bf = consts.tile([1, 128], BF16)
        nc.vector.memset(ones_bf, 1.0)
        ones_r = consts.tile([1, 64], F32R)
        nc.sync.dma_start(out=ones_r, in_=onesd[:])
        wtile = consts.tile([128, 512], BF16)
        nc.vector.memset(wtile, W_MASK)
        nskrow = consts.tile([1, 4, 256], F32, name="nskrow")
        for jj in range(4):
            nc.vector.memset(nskrow[:, jj, :], W_MASK * (S - KB * (jj + 1)))

        b2h_sb = consts.tile([128, 16], F32)
        nc.sync.dma_start(out=b2h_sb, in_=b2h[:])
        brow_sb = consts.tile([1, D], BF16)
        nc.sync.dma_start(out=brow_sb, in_=brow[:])
        bv512_sb = consts.tile([128, 8], F32)
        nc.sync.dma_start(out=bv512_sb, in_=bv512[:])
        bocol_sb = consts.tile([128, 8], F32)
        nc.sync.dma_start(out=bocol_sb, in_=bocol[:])
        mdup_sb = consts.tile([128, 4, 2, QT], U8)
        nc.sync.dma_start(out=mdup_sb, in_=mdup[:].rearrange("p (a h b) -> p a h b", a=4, h=2))

        wk_sb = consts.tile([128, NCH, D], BF16)
        wv_sb = consts.tile([128, NCH, D], BF16)
        wo_sb = consts.tile([128, NCH, D], BF16)
        O_sb = consts.tile([128, NCH, QL], BF16)
        QT_all = consts.tile([128, PAIRS, QL], BF16)
        xsum_sb = consts.tile([128, NCH, 4], BF16)

        with ExitStack() as ctx2:
            xt_pool = ctx2.enter_context(tc.tile_pool(name="xt", bufs=1))
            psum = ctx2.enter_context(tc.tile_pool(name="psum", bufs=1, space="PSUM"))

            xt_sb = xt_pool.tile([128, NCH, S], BF16)

            with tc.tile_pool(name="xqp", bufs=1) as xq_pool:
                xq_sb = xq_pool.tile([128, NCH, QL], BF16)
                wq_sb = xq_pool.tile([128, NCH, D], BF16)
                # DMA order = priority order
                for c in range(NCH):
                    nc.sync.dma_start(out=xq_sb[:, c, :], in_=xq[:, c, :])
                    nc.sync.dma_start(out=wq_sb[:, c, :], in_=wq[:, c, :])
                nc.sync.dma_start(out=wk_sb, in_=wk[:])
                for c in range(NCH):
                    nc.sync.dma_start(out=xt_sb[:, c, :], in_=xt[:, c, :])
                nc.sync.dma_start(out=wv_sb, in_=wv[:])
                nc.sync.dma_start(out=wo_sb, in_=wo[:])

                # PE warm-up while the x DMA streams in (results unused)
                warm = psum.tile([128, 512], F32, tag="pps", bufs=2, name="warm")
                for _ in range(36):
                    nc.tensor.matmul(out=warm, lhsT=wtile[:, 0:128],
                                     rhs=wtile[:, 0:512], start=True, stop=True)

                # Q projection for ALL pairs up front (only needs xq+wq)
                for pr in range(PAIRS):
                    for g2 in range(2):
                        ps = psum.tile([128, 512], F32, tag="pps", bufs=2, name="qp")
                        for c in range(NCH):
                            nc.tensor.matmul(
                                out=ps, lhsT=wq_sb[:, c, 128 * pr:128 * (pr + 1)],
                                rhs=xq_sb[:, c, 512 * g2:512 * (g2 + 1)],
                                start=(c == 0), stop=(c == NCH - 1),
                            )
                        nc.vector.tensor_scalar_add(
                            out=QT_all[:, pr, 512 * g2:512 * (g2 + 1)], in0=ps,
                            scalar1=b2h_sb[:, pr:pr + 1],
                        )

            vpool = ctx2.enter_context(tc.tile_pool(name="vsb", bufs=2))
            kt_pool = ctx2.enter_context(tc.tile_pool(name="kt", bufs=2))
            e_pool = ctx2.enter_context(tc.tile_pool(name="esb", bufs=4))
            bs_pool = ctx2.enter_context(tc.tile_pool(name="bs", bufs=2))
            misc_pool = ctx2.enter_context(tc.tile_pool(name="misc", bufs=2))

            # ---------------- fill-queue machinery ----------------
            fill = deque()
            kt_of = {}
            suf_of = {}
            v_of = {}

            def drain(n_mm):
                while n_mm > 0 and fill:
                    n_mm -= fill.popleft()()

            def flush():
                while fill:
                    fill.popleft()()

            def push_kproj(pr):
                KT_sb = kt_pool.tile([128, S], BF16, name="KT")
                kt_of[pr] = KT_sb

                def mk(kg):
                    def go():
                        ps = psum.tile([128, 512], F32, tag="pps", bufs=2, name="kp")
                        for c in range(NCH):
                            nc.tensor.matmul(
                                out=ps, lhsT=wk_sb[:, c, 128 * pr:128 * (pr + 1)],
                                rhs=xt_sb[:, c, 512 * kg:512 * (kg + 1)],
                                start=(c == 0), stop=(c == NCH - 1),
                            )
                        nc.vector.tensor_scalar_add(
                            out=KT_sb[:, 512 * kg:512 * (kg + 1)], in0=ps,
                            scalar1=b2h_sb[:, 8 + pr:9 + pr],
                        )
                        return NCH
                    return go
                for kg in range(4):
                    fill.append(mk(kg))

            def push_psb(pr):
                def go():
                    psb = psum.tile([128, 4], F32, tag="pps", bufs=2, name="psb")
                    for c in range(NCH):
                        nc.tensor.matmul(
                            out=psb, lhsT=wv_sb[:, c, 128 * pr:128 * (pr + 1)],
                            rhs=xsum_sb[:, c, :],
                            start=(c == 0), stop=(c == NCH - 1),
                        )
                    bs_sb = bs_pool.tile([128, 4], F32, tag="bs", name="bs_sb")
                    nc.vector.tensor_scalar(
                        out=bs_sb, in0=psb, scalar1=W_MASK,
                        scalar2=bv512_sb[:, pr:pr + 1], op0=ALU.mult, op1=ALU.add,
                    )
                    suf_sb = bs_pool.tile([128, 4], F32, tag="suf", name="suf_sb")
                    suf_of[pr] = suf_sb
                    nc.vector.memset(suf_sb[:, 3:4], 0.0)
                    nc.vector.tensor_copy(out=suf_sb[:, 2:3], in_=bs_sb[:, 3:4])
                    nc.vector.tensor_add(out=suf_sb[:, 1:2], in0=bs_sb[:, 2:3], in1=suf_sb[:, 2:3])
                    nc.vector.tensor_add(out=suf_sb[:, 0:1], in0=bs_sb[:, 1:2], in1=suf_sb[:, 1:2])
                    return NCH
                fill.append(go)

            def push_vproj(gp, direct_t=0):
                V_sb = vpool.tile([128, 16, 8, 65], BF16, name="V_sb")
                v_of[gp] = V_sb

                def ones_go():
                    nc.vector.memset(V_sb[:, :, :, 64], 1.0)
                    return 0
                if direct_t:
                    ones_go()
                else:
                    fill.append(ones_go)

                def mk(t):
                    def go():
                        ps = psum.tile([128, 512], F32, tag="pps", bufs=2, name="vp")
                        for c in range(NCH):
                            nc.tensor.matmul(
                                out=ps, lhsT=xt_sb[:, c, 128 * t:128 * (t + 1)],
                                rhs=wv_sb[:, c, 512 * gp:512 * (gp + 1)],
                                start=(c == 0), stop=False,
                            )
                        nc.tensor.matmul(
                            out=ps, lhsT=ones_bf,
                            rhs=brow_sb[:, 512 * gp:512 * (gp + 1)],
                            start=False, stop=True,
                        )
                        nc.vector.tensor_copy(
                            out=V_sb[:, t, :, 0:64],
                            in_=ps.rearrange("p (h d) -> p h d", h=8),
                        )
                        return NCH + 1
                    return go
                for t in range(direct_t):
                    mk(t)()
                for t in range(direct_t, 16):
                    fill.append(mk(t))

            # ---------------- bootstrap: pair 0 (+1) prereqs ----------------
            push_kproj(0)
            flush()
            # xsum on DVE after the K TS ops (keeps Q/K epilogues unblocked)
            for c in range(NCH):
                nc.vector.tensor_reduce(
                    out=xsum_sb[:, c, :],
                    in_=xt_sb[:, c, :].rearrange("p (b t) -> p b t", b=4),
                    axis=AX.X, op=ALU.add,
                )
            push_psb(0)
            push_vproj(0, direct_t=4)
            push_kproj(1)
            push_psb(1)

            # ---------------- main attention loop ----------------
            for pr in range(PAIRS):
                gp, lpi = pr // 4, pr % 4
                KT_sb, V_sb = kt_of[pr], v_of[gp]
                for J in range(2):
                    jlo, jhi = 2 * J, 2 * J + 1
                    po = [None, None]
                    for hl in range(2):
                        po[hl] = psum.tile([65, 512], F32, tag=f"po{hl}", bufs=1, name=f"po{hl}")
                    for kb in range(jhi + 1):
                        last = kb == jhi
                        dlo = kb == jlo
                        N = 256 if last else 512
                        qoff = 512 * J + (256 if last else 0)
                        for s2 in range(4):
                            pss = psum.tile([128, 2, 512], F32, tag="ss", bufs=2, name="ss")
                            k0 = 512 * kb + 128 * s2
                            for hl in range(2):
                                hsl = slice(64 * hl, 64 * (hl + 1))
                                nc.tensor.matmul(
                                    out=pss[:, hl, 0:N],
                                    lhsT=KT_sb[hsl, k0:k0 + 128],
                                    rhs=QT_all[hsl, pr, qoff:qoff + N],
                                    start=True, stop=True,
                                )
                            e_sb = e_pool.tile([128, 2, 512], BF16, tag="e", name="e_sb")
                            nc.scalar.activation(out=e_sb[:, :, 0:N], in_=pss[:, :, 0:N], func=AF.Exp)
                            if last or dlo:
                                nc.vector.copy_predicated(
                                    out=e_sb[:, :, 0:256],
                                    mask=mdup_sb[:, s2, :, :],
                                    data=wtile[:].rearrange("p (h b) -> p h b", h=2),
                                )
                            for hl in range(2):
                                nc.tensor.matmul(
                                    out=po[hl][:, qoff - 512 * J:qoff - 512 * J + N],
                                    lhsT=V_sb[:, 4 * kb + s2, 2 * lpi + hl, :],
                                    rhs=e_sb[:, hl, 0:N],
                                    start=(kb == 0 and s2 == 0),
                                    stop=(kb == jhi and s2 == 3),
                                    skip_group_check=True,
                                )
                            drain(1)
                    # epilogue: Z, broadcast, numerator correction, divide
                    suf_sb = suf_of[pr]
                    for hl in range(2):
                        hsl = slice(64 * hl, 64 * (hl + 1))
                        zfs = misc_pool.tile([1, 512], F32R, tag="zfs")
                        nc.vector.tensor_add(
                            out=zfs, in0=po[hl][64:65, 0:512],
                            in1=nskrow[:, 2 * J:2 * J + 2, :].rearrange("o a b -> o (a b)"),
                        )
                        zbc = psum.tile([64, 512], F32, tag="pps", bufs=2, name="zbc")
                        nc.tensor.matmul(out=zbc, lhsT=ones_r, rhs=zfs,
                                         start=True, stop=True)
                        rzb = misc_pool.tile([64, 512], F32, tag="rzb")
                        nc.vector.reciprocal_approx_fast(out=rzb, in_=zbc)
                        nm = misc_pool.tile([64, 512], F32, tag="nm")
                        for half, jj in ((0, jlo), (1, jhi)):
                            nc.vector.tensor_scalar_add(
                                out=nm[:, 256 * half:256 * (half + 1)],
                                in0=po[hl][0:64, 256 * half:256 * (half + 1)],
                                scalar1=suf_sb[hsl, jj:jj + 1],
                            )
                        oeng = nc.vector if hl == 0 else nc.gpsimd
                        oeng.tensor_mul(
                            out=O_sb[hsl, pr, 512 * J:512 * (J + 1)],
                            in0=nm, in1=rzb,
                        )
                        drain(4)
                flush()
                if pr + 2 < PAIRS:
                    push_kproj(pr + 2)
                    push_psb(pr + 2)
                if pr == 1:
                    push_vproj(1)

            # ---------------- output projection ----------------
            for dt_ in range(8):
                for J in range(2):
                    ps = psum.tile([128, 512], F32, tag="pps", bufs=2, name="fps")
                    for c in range(NCH):
                        nc.tensor.matmul(
                            out=ps, lhsT=wo_sb[:, c, 128 * dt_:128 * (dt_ + 1)],
                            rhs=O_sb[:, c, 512 * J:512 * (J + 1)],
                            start=(c == 0), stop=(c == NCH - 1),
                        )
                    fo = misc_pool.tile([128, 512], F32, tag="nm", name="fo")
                    nc.vector.tensor_scalar_add(out=fo, in0=ps, scalar1=bocol_sb[:, dt_:dt_ + 1])
                    nc.sync.dma_start(
                        out=outT[128 * dt_:128 * (dt_ + 1), 512 * J:512 * (J + 1)],
                        in_=fo,
                    )
    nc.compile()
    return nc


def qrows_for(p):
    return np.concatenate(
        [np.arange(QT * (2 * j + p), QT * (2 * j + p) + QT) for j in range(NJ)]
    )


def _bf16(a):
    import ml_dtypes
    return np.ascontiguousarray(a.astype(ml_dtypes.bfloat16))


def _chunked(mat2d, inner):
    return np.ascontiguousarray(mat2d.reshape(NCH, 128, inner).transpose(1, 0, 2))


def host_in_maps(x, Wqkv, bqkv, Wo, bo):
    x = np.asarray(x, np.float32)
    Wqkv = np.asarray(Wqkv, np.float32)
    bqkv = np.asarray(bqkv, np.float32)
    Wo = np.asarray(Wo, np.float32)
    bo = np.asarray(bo, np.float32)

    wq = _bf16(_chunked(Wqkv[:, 0:D] * 0.125, D))
    wk = _bf16(_chunked(Wqkv[:, D:2 * D], D))
    wv = _bf16(_chunked(Wqkv[:, 2 * D:3 * D], D))
    wo = _bf16(_chunked(Wo, D))

    b2h = np.empty((128, 16), np.float32)
    b2h[:, 0:8] = bqkv[0:D].reshape(8, 128).T / 8.0
    b2h[:, 8:16] = bqkv[D:2 * D].reshape(8, 128).T
    b2h = np.ascontiguousarray(b2h)
    brow = _bf16(bqkv[2 * D:].reshape(1, D))
    bv512 = np.ascontiguousarray((W_MASK * 512.0 * bqkv[2 * D:].reshape(8, 128)).T)
    bocol = np.ascontiguousarray(bo.reshape(8, 128).T)

    kap = np.arange(128)[:, None]
    r = np.arange(QT)[None, :]
    masks = {}
    for p in range(2):
        mm = np.zeros((128, 4, 1, QT), np.uint8)
        for s in range(4):
            mm[:, s, 0, :] = (128 * s + kap > QT * p + r)   # 1 = masked
        md = np.repeat(mm, 2, axis=2)                        # dup per head
        masks[p] = np.ascontiguousarray(md.reshape(128, 8 * QT))

    in_maps = []
    for core in range(8):
        b, p = core // 2, core % 2
        in_maps.append({
            "xq": _bf16(x[b][qrows_for(p)].T.reshape(NCH, 128, QL).transpose(1, 0, 2)),
            "xt": _bf16(x[b].T.reshape(NCH, 128, S).transpose(1, 0, 2)),
            "wq": wq, "wk": wk, "wv": wv, "wo": wo,
            "b2h": b2h, "brow": brow, "bv512": bv512, "bocol": bocol,
            "mdup": masks[p],
            "onesd": np.ones((1, 64), np.float32),
        })
    return in_maps


_CACHED = {}


def get_program():
    if "nc" not in _CACHED:
        _CACHED["nc"] = build_program()
    return _CACHED["nc"]


def kernel(x, Wqkv, bqkv, Wo, bo):
    from concourse.bass_utils import run_bass_kernel_spmd

    nc = get_program()
    in_maps = host_in_maps(x, Wqkv, bqkv, Wo, bo)
    res = run_bass_kernel_spmd(nc, in_maps, core_ids=list(range(8)))
    out = np.zeros((B, S, D), np.float32)
    for core in range(8):
        b, p = core // 2, core % 2
        out[b, qrows_for(p), :] = res.results[core]["outT"].T
    return out
